# revision 46
# baseline (speedup 1.0000x reference)
"""Trainium2 Bass kernel for single-query attention (nn_Attention_20040317403762).

Math (reassociated from the reference):
    q_b      = query_b @ Wq                       # [1, H]    (host, fp32)
    r_b      = Wk @ q_b^T / sqrt(H)               # [Din]     (host, fp32)
    scores_b = key_b @ r_b                        # [S]     (streams key once)
    attn_b   = softmax(scores_b)                  # online, no max-subtract
    u_b      = attn_b @ value_b                   # [Din]   (device)
    out_b    = (u_b / Z_b) @ Wv                   # [Dout]  (host, fp32)

v10 design (from v7):
  * 4-way column-tiled PE: every matmul here has M=2 (out partition dim 2),
    so it already runs in the 128x32 tiling mode - but v7 kept them all in
    tile T0, serializing the 512-cycle rhs streams. Placing outputs at PSUM
    partition bases 0/32/64/96 puts them in tiles T0-T3, which stream their
    own rhs via their own XBUS concurrently: ~4x PE throughput.
      - scores: chunk-pairs {2g,2g+1} (both batches, zero-padded r pairs)
        accumulate in tile g -> partial scores at partitions 32g+b. The fold
        happens in exp space: e = prod_g exp(sc_g) (4 ACT exps + 3 DVE muls,
        the last a tensor_tensor_reduce that also yields Z).
      - value: group g=(2b+h) accumulates u[b, 512h:512h+512] in tile g;
        all four live in ONE PSUM bank; host picks valid row b per group.
  * e pipeline: folded e [2,512] -> 4 blocked copies (rows 32c+b) -> one DVE
    32x32 block-transpose -> e in [s-partition, batch] layout = the lhsT of
    the value matmuls. No PE transposes anywhere.
  * host-side final projection: device returns unnormalized u + per-block
    softmax partial sums; host does (u/Z) @ Wv in fp32.
  * deadline-ordered single-ring DMA (all on SP/sync HWDGE): whole-block
    transfers only (each dma_start costs ~0.6us serialized descgen), order
    k0 k1 k2 v0 k3 v1 ... k7 v5 v6 v7.
  * PE warmup matmuls span the DMA ramp so the HAM clock-gate (3.4us
    activity window) is open before real data lands.

Dtypes: kmain (chunks 0-3) bf16, ktail (4-7) + value fp8 e3m4, e bf16,
PSUM fp32. Host numerics sim on the real data: rel_err 1.744e-2 (gate 2e-2).

Sharding: data-parallel over batch B=16 across 8 cores (2 batches/core).
"""

import sys

sys.path.insert(0, "/opt/trn_rl_repo")

import numpy as np
from contextlib import ExitStack

import concourse.bass as bass
import concourse.tile as tile
from concourse import bacc, mybir
from concourse.bass_utils import run_bass_kernel_spmd

FP = mybir.dt.float32
BF = mybir.dt.bfloat16
F16 = mybir.dt.float16
F8 = mybir.dt.float8e3

B = 16
S = 4096
D = 1024  # input dim == hidden dim == out dim
NCORES = 8
BPC = B // NCORES  # batches per core (paired)
P = 128
SB = 512            # s-block (PSUM bank width in fp32)
NCH = D // P        # 8 contraction chunks of the hidden dim
NT = S // P         # 32 s-tiles per batch
NB = S // SB        # 8 s-blocks per batch
TPB = SB // P       # 4 s-tiles per block
KSPLIT = 2          # keyT chunks 0..1 bf16; 2..7 e3m4 (sim: rel 1.940e-2)
NWARM = 14          # junk matmuls spanning the DMA ramp (HAM stays open)


def build_nc():
    nh = D // SB  # output halves (512-wide PSUM banks)

    nc = bacc.Bacc("TRN2", target_bir_lowering=False, debug=False)

    # pair-major key layout: [b, pair, p, block-in-pair, chunk, s] so a
    # 2-block pair is one dma_start with 2x-long descriptors
    km_d = nc.dram_tensor(
        "kmain", [BPC, NB // 2, P, 2, KSPLIT, SB], BF, kind="ExternalInput"
    ).ap()
    kt_d = nc.dram_tensor(
        "ktail", [BPC, NB // 2, P, 2, NCH - KSPLIT, SB], F8, kind="ExternalInput"
    ).ap()
    val_d = nc.dram_tensor("value", [BPC, P, NT, D], F8, kind="ExternalInput").ap()
    rp_d = nc.dram_tensor("rp", [P, BPC, NCH, 2], F16, kind="ExternalInput").ap()
    ou_d = nc.dram_tensor("out_u", [BPC, 2 * nh, SB], FP, kind="ExternalOutput").ap()
    oz_d = nc.dram_tensor("out_z", [BPC, NB], FP, kind="ExternalOutput").ap()

    with tile.TileContext(nc) as tc:
        with ExitStack() as ctx:
            singles = ctx.enter_context(tc.tile_pool(name="singles", bufs=1))
            kpool = ctx.enter_context(tc.tile_pool(name="kpool", bufs=NB))
            kfpool = ctx.enter_context(tc.tile_pool(name="kfpool", bufs=NB))
            vpool = ctx.enter_context(tc.tile_pool(name="vpool", bufs=NB))
            work = ctx.enter_context(tc.tile_pool(name="work", bufs=4))
            psum = ctx.enter_context(tc.tile_pool(name="psum", bufs=1, space="PSUM"))

            # ---- resident tiles ----
            rp_sb = singles.tile([P, BPC, NCH, 2], F16)  # padded r column pairs
            # blocked: e4[32c+b, bp, jj, q] = e_b[s=128jj+32c+q]; fp16 keeps
            # the exp-product fold's rounding out of the error budget
            # (max partial score ~3.8 -> exp ~45, far below fp16 max)
            e4 = singles.tile([P, NB, TPB, 32], F16)
            etT = singles.tile([P, NB, TPB, 32], F16)  # block-transposed
            zacc = singles.tile([BPC, NB], FP)
            junk_w = singles.tile([P, 2], BF)
            junk_r = singles.tile([P, SB], BF)
            warm_a = singles.tile([BPC, 32], BF)
            u_sb = singles.tile([BPC, 2 * nh, SB], FP)

            # ---- t=0: rp first on the ring; memsets + ACT table preload
            nc.sync.dma_start(rp_sb[:], rp_d)
            nc.vector.memset(junk_w[:], 0.0)
            nc.vector.memset(junk_r[:], 0.0)
            nc.vector.memset(e4[:], 0.0)
            nc.scalar.activation(
                warm_a[:], junk_r[0:BPC, 0:32], mybir.ActivationFunctionType.Exp
            )
            for _ in range(NWARM):
                w = psum.tile([P, TPB, 4, 32], FP, tag="scps", bufs=4)
                nc.tensor.matmul(
                    w[0:BPC, :, :, :], junk_w[:], junk_r[:], start=True, stop=True
                )

            # ---- stream DMAs, deadline order, single SP ring; 2-block
            # pairs per dma_start (longer descriptors), block-granular for
            # the head pair so the PE starts as early as possible ----
            k_tiles = {}   # (b, pair) -> (ktb [P,2,KSPLIT,SB], ktf)
            v_tiles = {}   # (b, pair) -> vt [P, 2*TPB, D]

            def issue_kp(pr, split=False):
                for b in range(BPC):
                    ktb = kpool.tile([P, 2, KSPLIT, SB], BF, tag="kq",
                                     name=f"k_{b}_{pr}")
                    ktf = kfpool.tile([P, 2, NCH - KSPLIT, SB], F8, tag="kf",
                                      name=f"kf_{b}_{pr}")
                    if split:
                        for i in range(2):
                            nc.sync.dma_start(ktb[:, i], km_d[b, pr, :, i])
                            nc.sync.dma_start(ktf[:, i], kt_d[b, pr, :, i])
                    else:
                        nc.sync.dma_start(ktb[:], km_d[b, pr])
                        nc.sync.dma_start(ktf[:], kt_d[b, pr])
                    k_tiles[(b, pr)] = (ktb, ktf)

            def issue_vp(pr, parts=1):
                # parts: how many dma_starts per batch (1=whole pair, 2=per
                # block, 4=half blocks) - finer at the tail so the last
                # accums never wait on bytes they don't need
                for b in range(BPC):
                    vt = vpool.tile([P, 2 * TPB, D], F8, tag="vs",
                                    name=f"v_{b}_{pr}")
                    t0 = pr * 2 * TPB
                    step = 2 * TPB // parts
                    for i in range(parts):
                        nc.sync.dma_start(
                            vt[:, i * step : (i + 1) * step, :],
                            val_d[b, :, t0 + i * step : t0 + (i + 1) * step, :],
                        )
                    v_tiles[(b, pr)] = vt

            issue_kp(0, split=True)   # blocks 0,1 at block granularity
            issue_kp(1)               # blocks 2,3
            issue_vp(0, parts=2)      # v0, v1
            issue_kp(2)               # blocks 4,5
            issue_vp(1)               # v2,3
            issue_kp(3)               # blocks 6,7
            issue_vp(2)               # v4,5
            issue_vp(3, parts=4)      # v6, v7 in half-blocks: earliest acc6/7

            # ---- compute ----
            # Column tiles: only T0-T2 are usable (quadrant 3 = cols 96-127
            # has a HW bug; bass restricts base_partition to {0,32,64}).
            # u accumulators (b,h): T0 carries two of them in two banks.
            u4a = psum.tile([P, SB], FP, tag="upsa", bufs=1)
            u4b = psum.tile([P, SB], FP, tag="upsb", bufs=1)
            # (b,h) -> (psum tile, column tile g); slot in u_sb stays 2b+h
            VMAP = {(0, 0): (u4a, 0), (0, 1): (u4a, 1), (1, 0): (u4a, 2),
                    (1, 1): (u4b, 0)}
            SGRP = [(0, 1), (2, 3, 4), (5, 6, 7)]  # score chunk groups

            def scores(bp):
                # tile g accumulates chunk group SGRP[g], both batches:
                # partial scores at partitions 32g+b, free = s (TPB,4,32)
                sc4 = psum.tile([P, TPB, 4, 32], FP, tag="scps", bufs=4,
                                name=f"sc_{bp}")
                for g, chunks in enumerate(SGRP):
                    og = sc4[32 * g : 32 * g + BPC, :, :, :]
                    mms = [(b, c) for c in chunks for b in range(BPC)]
                    for i, (b, c) in enumerate(mms):
                        ktb, ktf = k_tiles[(b, bp // 2)]
                        i2 = bp % 2
                        rhs = (ktb[:, i2, c, :] if c < KSPLIT
                               else ktf[:, i2, c - KSPLIT, :])
                        nc.tensor.matmul(
                            og,
                            rp_sb[:, b, c, :],
                            rhs,
                            start=(i == 0),
                            stop=(i == len(mms) - 1),
                            tile_position=(0, 32 * g),
                        )
                return sc4

            efin_t = {}

            def fold_a(bp, sc4):
                # e = prod_g exp(sc_g) in fp16; Z by separate reduce.
                # NOTE: tensor_tensor_reduce hangs TRN2 here (bisected on HW)
                ef = work.tile([BPC, 3, TPB, 4, 32], F16, tag="ef", bufs=4)
                m0 = work.tile([BPC, TPB, 4, 32], F16, tag="m0", bufs=4)
                efin = work.tile([BPC, TPB, 4, 32], F16, tag="efin", bufs=4)
                for g in range(3):
                    nc.scalar.activation(
                        ef[:, g, :, :, :],
                        sc4[32 * g : 32 * g + BPC, :, :, :],
                        mybir.ActivationFunctionType.Exp,
                    )
                nc.vector.tensor_mul(m0[:], ef[:, 0], ef[:, 1])
                nc.vector.tensor_mul(efin[:], m0[:], ef[:, 2])
                nc.vector.tensor_reduce(
                    zacc[:, bp : bp + 1], efin[:],
                    axis=mybir.AxisListType.XYZ, op=mybir.AluOpType.add,
                )
                efin_t[bp] = efin

            def fold_b(bp):
                # blocked layout (rows 32c+b) then 32x32 block-transpose;
                # emitted one block later so ACT's copies never head-of-line
                # block the next block's exps; copies split across ACT/DVE
                efin = efin_t.pop(bp)
                for c in range(4):
                    dst = e4[32 * c : 32 * c + BPC, bp, :, :]
                    if c % 2 == 0:
                        nc.scalar.copy(dst, efin[:, :, c, :])
                    else:
                        nc.vector.tensor_copy(dst, efin[:, :, c, :])
                nc.vector.transpose(etT[:, bp, :, :], e4[:, bp, :, :])

            def accum(bp):
                for jj in range(TPB):
                    lhsT = etT[:, bp, jj, 0:2]
                    for b in range(BPC):
                        vt = v_tiles[(b, bp // 2)]
                        jv = (bp % 2) * TPB + jj
                        for h in range(nh):
                            ups, g = VMAP[(b, h)]
                            nc.tensor.matmul(
                                ups[32 * g : 32 * g + BPC, :],
                                lhsT,
                                vt[:, jv, h * SB : (h + 1) * SB],
                                start=(bp == 0 and jj == 0),
                                stop=(bp == NB - 1 and jj == TPB - 1),
                                tile_position=(0, 32 * g),
                            )

            def keep_warm(n):
                # PE-idle gaps between DMA-paced blocks exceed the 3.4us HAM
                # window; junk matmuls in the gaps keep the clock at 2.4GHz
                wk = psum.tile([BPC, SB], FP, tag="warmps", bufs=1, name="wk")
                for _ in range(n):
                    nc.tensor.matmul(wk[:], junk_w[:], junk_r[:],
                                     start=True, stop=True)

            pending = []
            for bp in range(NB):
                sc4 = scores(bp)
                if len(pending) >= 2:
                    accum(pending.pop(0))
                keep_warm(8)
                if bp > 0:
                    fold_b(bp - 1)
                fold_a(bp, sc4)
                pending.append(bp)
            nc.sync.dma_start(oz_d, zacc[:])
            accum(pending.pop(0))
            fold_b(NB - 1)
            while pending:
                accum(pending.pop(0))

            # ---- tail: 4 group copies (2 engines), DMA per group so each
            # fires as soon as its copy lands ----
            for s, (b, h) in enumerate([(0, 0), (0, 1), (1, 0), (1, 1)]):
                ups, g = VMAP[(b, h)]
                src = ups[32 * g : 32 * g + BPC, :]
                if s % 2 == 0:
                    nc.vector.tensor_copy(u_sb[:, s, :], src)
                else:
                    nc.scalar.copy(u_sb[:, s, :], src)
                nc.sync.dma_start(ou_d[:, s : s + 1, :], u_sb[:, s : s + 1, :])

    nc.compile()
    return nc


_NC_CACHE = {}


def _get_nc():
    if "nc" not in _NC_CACHE:
        _NC_CACHE["nc"] = build_nc()
    return _NC_CACHE["nc"]


def make_in_maps(key, query, value, Wk, Wq, Wv, ncores=NCORES):
    import ml_dtypes

    bf16 = ml_dtypes.bfloat16
    f8 = ml_dtypes.float8_e3m4
    key = np.asarray(key, dtype=np.float32)
    query = np.ascontiguousarray(np.asarray(query, dtype=np.float32))
    value = np.asarray(value, dtype=np.float32)
    Wk = np.asarray(Wk, dtype=np.float32)
    Wq = np.asarray(Wq, dtype=np.float32)

    b = key.shape[0]
    # keyT blocks, partition-contiguous: kk[b, bp, p, c, s]; quantize each
    # stream straight from fp32 (no double rounding)
    kk32 = key.transpose(0, 2, 1).reshape(b, NCH, P, NB, SB).transpose(
        0, 3, 2, 1, 4
    )
    kmain = np.ascontiguousarray(kk32[:, :, :, 0:KSPLIT, :]).astype(bf16)
    ktail = np.ascontiguousarray(kk32[:, :, :, KSPLIT:NCH, :]).astype(f8)
    # pair-major: [b, pair, p, block-in-pair, chunk, s]
    kmain = np.ascontiguousarray(
        kmain.reshape(b, NB // 2, 2, P, KSPLIT, SB).transpose(0, 1, 3, 2, 4, 5)
    )
    ktail = np.ascontiguousarray(
        ktail.reshape(b, NB // 2, 2, P, NCH - KSPLIT, SB).transpose(0, 1, 3, 2, 4, 5)
    )
    # value partition-major: vshuf[b, p, t, d] = value[b, t*128 + p, d]
    vshuf = np.ascontiguousarray(
        value.reshape(b, NT, P, D).transpose(0, 2, 1, 3)
    ).astype(f8)
    # query-side prep (fp32): r_b = Wk @ (query_b @ Wq)^T / sqrt(H)
    q = query[:, 0, :] @ Wq                      # [B, H]
    r = (q @ Wk.T) / np.float32(np.sqrt(D))      # [B, Din]
    rcols = r.reshape(b, NCH, P).transpose(0, 2, 1).astype(np.float16)
    rp = np.zeros((b // BPC, P, BPC, NCH, 2), dtype=np.float16)
    for j in range(BPC):
        rp[:, :, j, :, j] = rcols[j::BPC]
    in_maps = []
    for c in range(ncores):
        sl = slice(c * BPC, (c + 1) * BPC)
        in_maps.append(
            {
                "kmain": kmain[sl],
                "ktail": ktail[sl],
                "value": vshuf[sl],
                "rp": rp[c],
            }
        )
    return in_maps


def run_sharded(inputs, trace=False, **kwargs):
    """Returns (full_output (B,1,D), BassKernelResults)."""
    in_maps = make_in_maps(**inputs)
    nc = _get_nc()
    res = run_bass_kernel_spmd(nc, in_maps, list(range(NCORES)), trace=trace, **kwargs)
    us, zs = [], []
    for i in range(NCORES):
        uu = np.asarray(res.results[i]["out_u"], dtype=np.float32)  # [BPC,4,SB]
        zz = np.asarray(res.results[i]["out_z"], dtype=np.float32)  # [BPC,NB]
        for b in range(BPC):
            # group g=2b+h holds u[b, h*SB:(h+1)*SB] in row b
            us.append(np.concatenate([uu[b, 2 * b + h, :] for h in range(2)]))
            zs.append(zz[b].sum())
    u = np.stack(us)          # [B, D]
    Z = np.array(zs, dtype=np.float32)
    Wv = np.asarray(inputs["Wv"], dtype=np.float32)
    out = (u / Z[:, None]) @ Wv
    return out.reshape(B, 1, D).astype(np.float32), res


def kernel(key, query, value, Wk, Wq, Wv):
    inputs = dict(key=key, query=query, value=value, Wk=Wk, Wq=Wq, Wv=Wv)
    try:
        out, _ = run_sharded(inputs)
        return out
    except Exception:
        # transient device failure: reset the backend and retry once
        import jax

        try:
            jax.clear_backends()
        except Exception:
            pass
        out, _ = run_sharded(inputs)
        return out
